# revision 10
# baseline (speedup 1.0000x reference)
"""CIEDE-base color-difference loss kernel for 8 Trainium2 NeuronCores.

Math (lightness_weight = 0, so L never matters):
  lin  = srgb_gamma(x)            -- CUSTOM DVE op: monic cubic Horner poly
                                     (L2 fit on [0,1]; scale folded into W1;
                                     end-to-end rel err ~8e-5)
  t    = (RGB2XYZ/white) @ lin    -- PE block-diag float32r matmul
  f    = cbrt(t) = exp(ln(t)/3)   -- ACT Ln (from PSUM, 1024-wide) + Exp
                                     (2048-wide); optionally some groups'
                                     Exp go to a CUSTOM DVE quartic poly
  da   = 500*((fx1-fy1)-(fx2-fy2));  db = 200*((fy1-fz1)-(fy2-fz2))  -- PE,
                                     two 512-chunks packed into [84, 512]
  s    = da^2 + db^2              -- CUSTOM DVE square (PSUM->SBUF bf16),
                                     then PE pair-sum W3p [84->42] packing
                                     3 sq-tiles into a [126, 512] s-tile
  cd   = sqrt(s) = exp(0.5*ln(s)) -- ACT (same ln/exp table set: no
                                     switches), free-dim-accumulated
  out  = mean over pixels         -- host-side f64 from per-partition partials

Layout: batch data-parallel, 4 image-pairs per core. Each image plane is
host-padded to 21 partition-rows x 12544 cols (pad = 0.5 in both images so
padded pixels contribute cd ~ 0). Partitions are channel-blocked (p = 21*c+k,
c in r1,g1,b1,r2,g2,b2), so each [126, F] tile holds 21*F pixel pairs, loads
as two contiguous [63, F] DMAs (one per image), and every cross-channel op is
a kron(A, I21) matmul on the otherwise-idle PE.
"""

import numpy as np
import ml_dtypes

B, C, H, W = 32, 3, 512, 512
HWPX = H * W                 # 262144 pixels per image
N_CORES = 8
B_LOC = B // N_CORES         # 4 image-pairs per core
ROWS = 21                    # partition-rows per image
ROWL = 12544                 # cols per partition-row; host pads each plane to
PADPX = ROWS * ROWL          # 263424 px (+1280 pad px, value 0.5 both images)
FULL_F = 2048                # free-dim per full tile group
N_FULL = 6                   # full groups per image pair (6*2048 = 12288)
RAG_F = ROWL - N_FULL * FULL_F   # 256 ragged cols

# which full groups' cbrt-Exp stage runs on the DVE (quartic poly) instead of
# the ACT engine; tunable load balance. () = all on ACT.
EXP_DVE_GROUP_IDS = (1, 4)

_RGB2XYZ = np.array([[0.4124564, 0.3575761, 0.1804375],
                     [0.2126729, 0.7151522, 0.0721750],
                     [0.0193339, 0.1191920, 0.9503041]], dtype=np.float64)
_WHITE = np.array([0.95047, 1.0, 1.08883], dtype=np.float64)

bf16 = ml_dtypes.bfloat16


def _gamma_cubic():
    """L2 fit of the exact (piecewise) sRGB gamma by a cubic on [0,1].
    Returns (s, a2, a1, a0): gamma(x) ~ s * (((x+a2)x+a1)x+a0)."""
    xs = np.linspace(0.0, 1.0, 200001)
    ys = np.where(xs <= 0.04045, xs / 12.92, ((xs + 0.055) / 1.055) ** 2.4)
    c = np.polyfit(xs, ys, 3)          # c[0]x^3 + ... + c[3]
    s = float(c[0])
    return s, float(c[1] / c[0]), float(c[2] / c[0]), float(c[3] / c[0])


def _exp_quartic():
    """L2 fit of exp(y/3) by a quartic on y in [-6.3, 0.01] (y = ln t domain).
    Returns (s4, a3, a2, a1, a0): exp(y/3) ~ s4 * ((((y+a3)y+a2)y+a1)y+a0)."""
    ys = np.linspace(-6.3, 0.01, 200001)
    c = np.polyfit(ys, np.exp(ys / 3.0), 4)
    s4 = float(c[0])
    return s4, *(float(ci / c[0]) for ci in c[1:])


GAM_S, GAM_A2, GAM_A1, GAM_A0 = _gamma_cubic()
EXP_S, EXP_A3, EXP_A2, EXP_A1, EXP_A0 = _exp_quartic()


def _patch_act_tables():
    """Reorder the activation-table list so `natural_log_exp_and_others`
    (which contains BOTH Ln and Exp) is the first match for either function.
    Without this the table-load inserter picks exp_and_others for Exp and
    natural_log for Ln, emitting a ~1.3us ACT_TABLE_LOAD at every Ln<->Exp
    transition (66 of them = 85us of ACT time in this kernel)."""
    import concourse.bacc as bacc_mod
    from concourse import mybir as mb
    if getattr(bacc_mod, "_ant_cd_tables_patched", False):
        return
    orig = bacc_mod.get_activation_tables

    def filtered(arch):
        # Set order/indices must stay intact (walrus remaps the emitted
        # act_func_set_id positionally), so instead of reordering, hide Exp
        # from exp_and_others and Ln from natural_log: the first set
        # containing either function is then natural_log_exp_and_others.
        t = orig(arch)
        t = {k: set(v) for k, v in t.items()}
        t.get("exp_and_others", set()).discard(mb.ActivationFunctionType.Exp)
        t.get("natural_log", set()).discard(mb.ActivationFunctionType.Ln)
        return t

    bacc_mod.get_activation_tables = filtered
    bacc_mod._ant_cd_tables_patched = True


def _register_dve_ops():
    """Register the custom DVE ops (documented extension point: append to
    dve_ops.OPS; per-NEFF table is generated for ops actually emitted)."""
    from concourse.dve_spec import Spec, Src0, C0, C1, C2, C3, lower, sq
    from concourse.dve_spec import _spill_c3_to_src1
    from concourse.dve_uop import DveOpSpec
    from concourse import dve_ops

    def make(name, spec, rd1):
        uops = lower(spec, ver="v3")
        sha = DveOpSpec(name=name, opcode=1, uops=uops, rd1_en=rd1).sha("v3")
        op = dve_ops.DveOp(name, spec, subdim=False, uops_sha={"v3": sha})
        dve_ops.OPS.append(op)
        dve_ops._SUB_OPCODE_FOR_NAME[name] = (
            dve_ops._CUSTOM_DVE_ROW_BASE + len(dve_ops.OPS) - 1)
        dve_ops.CUSTOM_DVE_SPECS[name] = spec
        return op

    byname = {op.name: op for op in dve_ops.OPS}
    ops = {}
    if "ANT_CD_POLY3" in byname:
        ops["poly3"] = byname["ANT_CD_POLY3"]
        ops["square"] = byname["ANT_CD_SQUARE"]
        ops["poly4"] = byname["ANT_CD_POLY4"]
        return ops
    # monic cubic Horner: ((x + C0)x + C1)x + C2
    ops["poly3"] = make(
        "ANT_CD_POLY3",
        Spec(body=((Src0 + C0) * Src0 + C1) * Src0 + C2,
             reference=lambda in0, s0, s1, imm2:
                 (((in0 + s0) * in0 + s1) * in0 + imm2).astype(np.float32)),
        rd1=False)
    # elementwise square (PSUM fp32 -> SBUF bf16 conversion in the same op)
    ops["square"] = make(
        "ANT_CD_SQUARE",
        Spec(body=sq(Src0),
             reference=lambda in0, s0, s1, imm2: (in0 * in0)),
        rd1=False)
    # monic quartic Horner: (((x + C0)x + C1)x + C2)x + C3  (C3 via in1)
    ops["poly4"] = make(
        "ANT_CD_POLY4",
        Spec(body=_spill_c3_to_src1(
                 (((Src0 + C0) * Src0 + C1) * Src0 + C2) * Src0 + C3),
             reference=lambda in0, in1, s0, s1, imm2:
                 ((((in0 + s0) * in0 + s1) * in0 + imm2) * in0
                  + in1).astype(np.float32)),
        rd1=True)
    return ops


def _build_weights():
    """Channel-blocked layout: partition p = 21*c + k (c = channel slot 0..5
    meaning r1,g1,b1,r2,g2,b2; k = pixel row 0..20). All block-diagonal maps
    become kron(A, I21)."""
    f32 = np.float32
    I21 = np.eye(ROWS, dtype=f32)
    Mp = (_RGB2XYZ / _WHITE[:, None]).astype(np.float64)   # white folded in
    M6 = np.zeros((6, 6), np.float64)
    M6[:3, :3] = Mp
    M6[3:, 3:] = Mp
    # stage 1: out = lhsT.T @ rhs; gamma poly scale folded in here.
    W1 = np.kron(M6.T * GAM_S, I21).astype(f32)             # [126, 126]
    # stage 2: da rows 0..20, db rows 21..41; coeffs exact in bf16. Padded to
    # 64 output rows (cols 42:64 zero) so the [128, F] dd tile's unused rows
    # are written 0.0 (PE tile positions must be 32-aligned; garbage rows
    # would turn into NaN via the 0-weight W3E columns otherwise).
    A = np.array([[500.0, -500.0, 0.0, -500.0, 500.0, 0.0],
                  [0.0, 200.0, -200.0, 0.0, -200.0, 200.0]], f32)
    W2z = np.zeros((126, 64), f32)
    W2z[:, :42] = np.kron(A.T, I21)
    # stage 3 pair-sum for a [128, F] sq tile holding chunk A at rows 0:42
    # and chunk B at rows 64:106: out[42*slot+c] = sq[c]+sq[21+c] (A),
    # out[42*slot+21+c] = sq[64+c]+sq[85+c] (B); slot = position in the
    # [126, F] s accumulator (3 sq-tiles per s-tile via start/stop chain).
    W3E = np.zeros((128, 3 * 126), f32)
    for slot in range(3):
        E = W3E[:, 126*slot:126*(slot+1)]
        for c in range(ROWS):
            E[c, 42*slot+c] = 1.0
            E[21+c, 42*slot+c] = 1.0
            E[64+c, 42*slot+21+c] = 1.0
            E[85+c, 42*slot+21+c] = 1.0
    W3d = np.kron(np.ones((2, 1), f32), I21)                # [42, 21]
    return W1, W2z.astype(bf16), W3E.astype(bf16), W3d.astype(bf16)


_CACHE = {}
PROD_VARIANT = "fullp"   # software-prefetch dual-ring issue (see issue_loads)


def _build_module(reps=1, variant="full", unroll=1):
    """variant: 'full' | 'dma' (loads + tiny DVE reduce).
    unroll: repeat the whole 4-pair body inside the reps loop (bench-only;
    per-rep = unroll*exec + loop-barrier, so two unroll values solve for
    exec with the barrier cancelled)."""
    import concourse.bass as bass
    import concourse.bacc as bacc
    import concourse.tile as tile
    from concourse import mybir

    _patch_act_tables()
    dve = _register_dve_ops()

    f32 = mybir.dt.float32
    bft = mybir.dt.bfloat16
    AF = mybir.ActivationFunctionType

    nc = bacc.Bacc(None, target_bir_lowering=False)

    img1h = nc.dram_tensor("img1", [B_LOC, C, PADPX], f32, kind="ExternalInput")
    img2h = nc.dram_tensor("img2", [B_LOC, C, PADPX], f32, kind="ExternalInput")
    f32r = mybir.dt.float32r
    w1h = nc.dram_tensor("w1", [126, 126], f32r, kind="ExternalInput")
    w2h = nc.dram_tensor("w2", [126, 64], bft, kind="ExternalInput")
    w3eh = nc.dram_tensor("w3e", [128, 3 * 126], bft, kind="ExternalInput")
    w3dh = nc.dram_tensor("w3d", [42, ROWS], bft, kind="ExternalInput")
    outh = nc.dram_tensor("partials", [126, 2 * B_LOC], f32, kind="ExternalOutput")

    imgs = [img1h, img2h]

    split = bool(EXP_DVE_GROUP_IDS) and variant.startswith("full")
    # with the exp split active, every f is scaled by 1/EXP_S (ACT Exp gets
    # bias -ln(EXP_S); the DVE quartic is monic); host multiplies by |EXP_S|.
    exp_bias = -float(np.log(EXP_S)) if split else 0.0

    HALF_F = 3 * FULL_F          # x loads at half-pair granularity (1.55 MB
    # per DMA: big transfers amortize the ~2us per-DMA fixed cost; img1 goes
    # through the sync HWDGE queue, img2 through the gpsimd SWDGE queue so
    # the two streams drain in parallel)

    def dram_src3(img_idx, b, col0, ncols):
        """[3 channels x 21 rows, ncols] of image b: one DMA's worth."""
        h = imgs[img_idx]
        off = b * C * PADPX + col0
        return bass.AP(tensor=h, offset=off,
                       ap=[[PADPX, C], [ROWL, ROWS], [1, ncols]])

    with tile.TileContext(nc) as tc:
        from contextlib import ExitStack
        with ExitStack() as ctx:
            singles = ctx.enter_context(tc.tile_pool(name="singles", bufs=1))
            xpool = ctx.enter_context(tc.tile_pool(name="x", bufs=4))
            xrpool = ctx.enter_context(tc.tile_pool(name="xr", bufs=2))
            linpool = ctx.enter_context(tc.tile_pool(name="lin", bufs=3))
            ltpool = ctx.enter_context(tc.tile_pool(name="lt", bufs=3))
            fpool = ctx.enter_context(tc.tile_pool(name="f", bufs=3))
            sqpool = ctx.enter_context(tc.tile_pool(name="sq", bufs=3))
            qpool = ctx.enter_context(tc.tile_pool(name="q", bufs=2))
            # PSUM bank budget (8 banks):
            #   t [126,1024] x2 bufs = 4, dd [84,512] x2 = 2, s [126,512] x2 = 2
            tpool = ctx.enter_context(tc.tile_pool(name="t", bufs=2, space="PSUM"))
            ddpool = ctx.enter_context(tc.tile_pool(name="dd", bufs=2, space="PSUM"))
            spool = ctx.enter_context(tc.tile_pool(name="s", bufs=2, space="PSUM"))

            w1f = singles.tile([126, 126], f32r)
            w2 = singles.tile([126, 64], bft)
            w3e = singles.tile([128, 3 * 126], bft)
            w3d = singles.tile([42, ROWS], bft)
            # fullp: weights ride the scalar ring so the sync ring's first
            # img1 DMA starts at t=0 (weights are tiny and ACT is idle then)
            weng = nc.scalar if variant == "fullp" else nc.sync
            weng.dma_start(out=w1f[:], in_=w1h[:, :])
            weng.dma_start(out=w2[:], in_=w2h[:, :])
            weng.dma_start(out=w3e[:], in_=w3eh[:, :])
            weng.dma_start(out=w3d[:], in_=w3dh[:, :])

            acc = singles.tile([126, 2 * B_LOC], f32)
            nc.vector.memset(acc[:], 0.0)

            ebias = singles.tile([128, 1], f32)
            nc.vector.memset(ebias[:], 1e-35)
            fbias = singles.tile([128, 1], f32)
            nc.vector.memset(fbias[:], exp_bias)
            e4bias = singles.tile([128, 1], f32)
            nc.vector.memset(e4bias[:], EXP_A0)

            xconst = None
            if variant == "nodma2":
                # compute-wall diagnostic: persistent x tiles written once
                # outside the reps loop; the loop body is pure consumer
                xc0 = singles.tile([126, HALF_F], f32)
                xc1 = singles.tile([126, HALF_F], f32)
                xc2 = singles.tile([126, RAG_F], f32)
                xconst = (xc0, xc1, xc2)
                for xc in xconst:
                    nc.vector.memset(xc[:], 0.5)

            # issue rings: img1 -> sync HWDGE, img2 -> scalar HWDGE: the two
            # rings together sustain ~193 GB/s vs ~173 single (per-core HBM
            # cap); SWDGE (gpsimd) measured slower, don't use it.
            issuers = [nc.sync,
                       nc.gpsimd if variant == "dma"
                       else nc.sync if variant in ("dma_sync", "fulls")
                       else nc.scalar]

            def issue_loads(b):
                """Allocate + DMA one pair's x tiles (ragged first)."""
                xr = xrpool.tile([126, RAG_F], f32, tag="xr")
                for ii in range(2):
                    issuers[ii].dma_start(
                        out=xr[63*ii:63*ii+63, :],
                        in_=dram_src3(ii, b, N_FULL * FULL_F, RAG_F))
                xh = []
                for hh in range(2):
                    xt = xpool.tile([126, HALF_F], f32, tag="x")
                    for ii in range(2):
                        issuers[ii].dma_start(
                            out=xt[63*ii:63*ii+63, :],
                            in_=dram_src3(ii, b, hh * HALF_F, HALF_F))
                    xh.append(xt)
                return xh, xr

            if reps > 1:
                loop_cm = tc.For_i(0, reps, 1)
                loop_cm.__enter__()

            seq = [bb for _ in range(unroll) for bb in range(B_LOC)]
            # fullp: software prefetch — pair b+1's loads are traced BEFORE
            # pair b's compute, so the scalar-ring dma_starts sit ahead of
            # pair b's ACT work in the ACT FIFO and issue immediately
            # (instead of serializing behind ~25us of activations per pair).
            prefetch = variant == "fullp"
            pend = issue_loads(seq[0]) if prefetch else None

            for idx, b in enumerate(seq):
                # sqrt Ln outputs for the 4 packed s-tiles of this pair
                q = qpool.tile([126, 4 * 512], f32, tag="q")
                s_cur = None
                only1 = variant == "dma1"
                nodma = variant == "nodma"
                if prefetch:
                    xh, xr = pend
                    if idx + 1 < len(seq):
                        pend = issue_loads(seq[idx + 1])
                elif variant == "nodma2":
                    xh = [xconst[0], xconst[1]]
                    xr = xconst[2]
                else:
                    xr = xrpool.tile([126, RAG_F], f32, tag="xr")
                    for ii in range(0 if nodma else 1 if only1 else 2):
                        issuers[ii].dma_start(
                            out=xr[63*ii:63*ii+63, :],
                            in_=dram_src3(ii, b, N_FULL * FULL_F, RAG_F))
                    if nodma:
                        # compute-wall diagnostic: x produced by the
                        # (otherwise idle) Pool engine instead of DMA
                        nc.gpsimd.memset(xr[:], 0.5)
                    xh = []
                    for hh in range(2):
                        xt = xpool.tile([126, HALF_F], f32, tag="x")
                        if nodma:
                            nc.gpsimd.memset(xt[:], 0.5)
                        elif only1:
                            # half the bytes, still dual-queue (split by half)
                            issuers[hh].dma_start(
                                out=xt[0:63, :],
                                in_=dram_src3(0, b, hh * HALF_F, HALF_F))
                        elif variant == "dma3q":
                            # img1 -> sync, img2 A -> scalar, B -> swdge
                            nc.sync.dma_start(
                                out=xt[0:63, :],
                                in_=dram_src3(0, b, hh * HALF_F, HALF_F))
                            eng2 = nc.scalar if hh == 0 else nc.gpsimd
                            eng2.dma_start(
                                out=xt[63:126, :],
                                in_=dram_src3(1, b, hh * HALF_F, HALF_F))
                        else:
                            for ii in range(2):
                                issuers[ii].dma_start(
                                    out=xt[63*ii:63*ii+63, :],
                                    in_=dram_src3(ii, b, hh * HALF_F, HALF_F))
                        xh.append(xt)

                if variant.startswith("dma"):
                    # tiny consumer per tile (keeps pool recycling honest
                    # without adding measurable DVE time)
                    for xt in xh + [xr]:
                        red = qpool.tile([126, 1], f32, tag="red")
                        nc.vector.tensor_reduce(
                            out=red[:], in_=xt[:, :8], op=mybir.AluOpType.max,
                            axis=mybir.AxisListType.X)
                    continue

                # ragged group first: its small ops fill scheduling gaps
                for gg in [N_FULL] + list(range(N_FULL)):
                    ragged = gg == N_FULL
                    F = RAG_F if ragged else FULL_F

                    if ragged:
                        x = xr[:, :]
                    else:
                        x = xh[gg // 3][:, (gg % 3) * FULL_F:
                                        (gg % 3) * FULL_F + FULL_F]

                    # gamma: monic cubic on the DVE (scale folded into W1)
                    lin = linpool.tile([126, F], f32r, tag="lin")
                    nc.vector._custom_dve(dve["poly3"], out=lin[:], in0=x[:],
                                          s0=GAM_A2, s1=GAM_A1, imm2=GAM_A0)

                    # stage 1 matmuls into [126, 1024] PSUM tiles; ACT Ln
                    # reads each tile in one instruction
                    lt = ltpool.tile([126, F], f32, tag="lt")
                    nch = (F + 1023) // 1024
                    for h in range(nch):
                        c0 = h * 1024
                        cw = min(1024, F - c0)
                        tq = tpool.tile([126, cw], f32, tag="t")
                        for hh in range(0, cw, 512):
                            hw_ = min(512, cw - hh)
                            nc.tensor.matmul(tq[:, hh:hh+hw_], w1f[:],
                                             lin[:, c0+hh:c0+hh+hw_],
                                             start=True, stop=True)
                        # cbrt part 1: lt = ln(t) straight from PSUM
                        nc.scalar.activation(out=lt[:, c0:c0+cw], in_=tq[:],
                                             func=AF.Ln)
                    # cbrt part 2: f = exp(lt/3) as bf16 (ACT or DVE quartic)
                    f = fpool.tile([126, F], bft, tag="f")
                    if not ragged and gg in EXP_DVE_GROUP_IDS:
                        nc.vector._custom_dve(
                            dve["poly4"], out=f[:], in0=lt[:],
                            in1=e4bias[0:126], s0=EXP_A3, s1=EXP_A2,
                            imm2=EXP_A1)
                    else:
                        nc.scalar.activation(out=f[:], in_=lt[:], func=AF.Exp,
                                             scale=float(1.0 / 3.0),
                                             bias=fbias[0:126])

                    # stage 2 (da,db): two 512-chunks land at rows 0:64 and
                    # 64:128 of a [128,512] dd tile (W2 pads rows 42:64 with
                    # zeros), square on DVE, then stage 3 pair-sum packs 3
                    # sq-tiles densely into a [126,512] s accumulator via the
                    # shifted W3E blocks (start/stop chain).
                    if ragged:
                        dd = ddpool.tile([64, RAG_F], f32, tag="dd")
                        nc.tensor.matmul(dd[:], w2[:], f[:],
                                         start=True, stop=True)
                        sqr = sqpool.tile([64, RAG_F], bft, tag="sqr")
                        nc.vector._custom_dve(dve["square"], out=sqr[:],
                                              in0=dd[:])
                        srag = spool.tile([ROWS, RAG_F], f32, tag="s")
                        nc.tensor.matmul(srag[:], w3d[:], sqr[0:42, :],
                                         start=True, stop=True)
                        # sqrt of ragged s; accumulate into acc col 2b+1
                        qr = qpool.tile([ROWS, RAG_F], f32, tag="qrag")
                        nc.scalar.activation(out=qr[:], in_=srag[:], func=AF.Ln,
                                             bias=ebias[0:ROWS])
                        nc.scalar.activation(out=qr[:], in_=qr[:], func=AF.Exp,
                                             scale=0.5,
                                             accum_out=acc[0:ROWS, 2*b+1:2*b+2])
                        continue

                    for jj in range(2):
                        dd = ddpool.tile([128, 512], f32, tag="dd")
                        for u in range(2):
                            fc = jj * 1024 + u * 512
                            nc.tensor.matmul(dd[64*u:64*u+64, :], w2[:],
                                             f[:, fc:fc+512],
                                             start=True, stop=True)
                        sqt = sqpool.tile([128, 512], bft, tag="sq")
                        nc.vector._custom_dve(dve["square"], out=sqt[:],
                                              in0=dd[:])
                        k = gg * 2 + jj          # sq-tile index 0..11
                        slot = k % 3
                        if slot == 0:
                            s_cur = spool.tile([126, 512], f32, tag="s")
                        nc.tensor.matmul(s_cur[:],
                                         w3e[:, 126*slot:126*slot+126], sqt[:],
                                         start=(slot == 0), stop=(slot == 2))
                        if slot == 2:
                            j = k // 3
                            nc.scalar.activation(out=q[:, 512*j:512*j+512],
                                                 in_=s_cur[:], func=AF.Ln,
                                                 bias=ebias[0:126])

                if variant.startswith("dma"):
                    continue
                # sqrt part 2 over the pair's 4 packed s-tiles
                nc.scalar.activation(out=q[:], in_=q[:], func=AF.Exp,
                                     scale=0.5, accum_out=acc[:, 2*b:2*b+1])

            if reps > 1:
                loop_cm.__exit__(None, None, None)

            nc.sync.dma_start(out=outh[:, :], in_=acc[:])

    nc.compile()
    return nc


def _get_module(reps=1, variant="full", unroll=1):
    key = f"nc{reps}_{variant}_{unroll}"
    if key not in _CACHE:
        _CACHE[key] = _build_module(reps, variant, unroll)
    return _CACHE[key]


def make_in_maps(img1, img2):
    img1 = np.asarray(img1)
    img2 = np.asarray(img2)
    w1, w2, w3e, w3d = _build_weights()
    in_maps = []
    for d in range(N_CORES):
        sl = slice(d * B_LOC, (d + 1) * B_LOC)
        m = {"w1": w1, "w2": w2, "w3e": w3e, "w3d": w3d}
        for name, img in (("img1", img1), ("img2", img2)):
            pad = np.full((B_LOC, C, PADPX), 0.5, np.float32)
            pad[:, :, :HWPX] = img[sl].reshape(B_LOC, C, HWPX)
            m[name] = pad
        in_maps.append(m)
    return in_maps


def kernel(img1, img2):
    import concourse.bass_utils as bass_utils

    img1 = np.ascontiguousarray(np.asarray(img1), dtype=np.float32)
    img2 = np.ascontiguousarray(np.asarray(img2), dtype=np.float32)
    assert img1.shape == (B, C, H, W)

    nc = _get_module(variant=PROD_VARIANT)
    in_maps = make_in_maps(img1, img2)

    res = bass_utils.run_bass_kernel_spmd(nc, in_maps, core_ids=list(range(N_CORES)))
    _CACHE["last_results"] = res

    scale = abs(EXP_S) if EXP_DVE_GROUP_IDS else 1.0
    out = np.empty(B, dtype=np.float32)
    for d in range(N_CORES):
        acc = res.results[d]["partials"].astype(np.float64)  # [126, 8]
        for b in range(B_LOC):
            total = acc[:, 2*b].sum() + acc[:ROWS, 2*b+1].sum()
            out[d * B_LOC + b] = total * scale / HWPX
    return out


if __name__ == "__main__":
    i1 = np.load("/root/problem/img1.npy")
    i2 = np.load("/root/problem/img2.npy")
    print(kernel(i1, i2))

